# revision 10
# baseline (speedup 1.0000x reference)
"""Trainium2 Bass kernel for MultiHeadAttentionXL (Transformer-XL attention).

Problem: input_ (S=1024, B=8, D=1024), 16 heads x 64 dim, causal mask.
Sharding: data-parallel over batch -- one batch element per NeuronCore (8 cores).

Per-core algorithm (all matmuls f32r: 1 cycle/row on PE, ~1e-4 accuracy):
  xT (D, S) resident in SBUF (host pre-transposed).
  v = x @ W_v             -> v' tiles (s-part, [16 heads x (64 d + ones col)])
  q = x @ W_q  (as qT)    -> quv_h tiles (128 = [q_h+u; q_h+v], S) via psum evac
  k = x @ W_k  (as kT)    -> khp_h tiles rows 0:64
  p = pos @ W_p (as pT)   -> khp_h tiles rows 64:128
  scores^T[j,i] = khp_h[:,j].T @ quv_h[:,i]   (one K=128 matmul per j-tile)
  attnT = exp(SCALE * scores^T + mask)        (ACT, additive mask blocks on DVE)
  avT[dd,i]   = v'_h[j,dd].T @ attnT[j,i]     (accumulated over j-tiles; row 64
                                               of v' is ones -> denominator row)
  awvT = avT[0:64] * recip(avT[64])           (softmax normalization)
  out = awvT.T @ W_out
Block skipping and additive-mask patterns are derived from the actual mask
tensor at compile time (optimal for causal, correct for any mask).
"""

import sys

if "/opt/trn_rl_repo" not in sys.path:
    sys.path.insert(0, "/opt/trn_rl_repo")

import numpy as np

import concourse.bass as bass
import concourse.mybir as mybir
import concourse.tile as tile
from concourse import bacc
from concourse.bass_utils import run_bass_kernel_spmd

S = 1024
B = 8
D = 1024
H = 16
DH = 64
SCALE = 1.0 / (DH**0.5)
P = 128            # partitions / tile edge
KC = D // P        # 8 contraction chunks
NMB = H // 2       # 8 head-pairs (m-blocks of 128)
NSC = 2            # i-chunks of 512
SC = 512
NJT = S // P       # 8 j-tiles
NEG = -1e30

F32 = mybir.dt.float32
F32R = mybir.dt.float32r

_BUILD_CACHE: dict = {}
_ONES = np.ones((128, 16), np.float32)


def _analyze_mask(mask: np.ndarray):
    """Derive block structure from the (S, S, 1) bool mask.

    Returns (struct, patterns) where struct[c] is a list of
    (jt, ((it_local, pat_idx), ...)) for each non-fully-masked j-tile of
    chunk c, and patterns is an (NP, 128, 128) f32 array of unique additive
    mask blocks in (j, i) orientation.
    """
    m = np.asarray(mask).reshape(S, S)
    pat_map: dict[bytes, int] = {}
    patterns: list[np.ndarray] = []
    struct = []
    for c in range(NSC):
        jts = []
        for jt in range(NJT):
            ops = []
            n_full = 0
            for il in range(4):
                it = c * 4 + il
                blk = m[it * P:(it + 1) * P, jt * P:(jt + 1) * P]
                if not blk.any():
                    continue
                if blk.all():
                    n_full += 1
                pat = (NEG * blk.T).astype(np.float32)  # (j, i) orientation
                key = pat.tobytes()
                if key not in pat_map:
                    pat_map[key] = len(patterns)
                    patterns.append(pat)
                ops.append((il, pat_map[key]))
            if n_full == 4:
                continue  # whole j-tile masked for this chunk: skip
            jts.append((jt, tuple(ops)))
        struct.append(tuple(jts))
    if not patterns:
        patterns.append(np.zeros((P, P), np.float32))
    return tuple(struct), np.stack(patterns)


def _build(struct, n_pats: int):
    nc = bacc.Bacc("TRN2", target_bir_lowering=False, debug=False, num_devices=B,
                   dynamic_dma_scratch_size=2048)

    xT_d = nc.declare_dram_parameter("xT", [D, S], F32, isOutput=False)
    posT_d = nc.declare_dram_parameter("posT", [D, S], F32, isOutput=False)
    wq_d = nc.declare_dram_parameter("Wq", [D, H * DH], F32, isOutput=False)
    wk_d = nc.declare_dram_parameter("Wk", [D, H * DH], F32, isOutput=False)
    wv_d = nc.declare_dram_parameter("Wv", [D, H * DH], F32, isOutput=False)
    wp_d = nc.declare_dram_parameter("Wp", [D, H * DH], F32, isOutput=False)
    wo_d = nc.declare_dram_parameter("Wout", [H * DH, D], F32, isOutput=False)
    ucol_d = nc.declare_dram_parameter("ucol", [H * DH], F32, isOutput=False)
    vcol_d = nc.declare_dram_parameter("vcol", [H * DH], F32, isOutput=False)
    mp_d = nc.declare_dram_parameter("mpats", [n_pats, P, P], F32, isOutput=False)
    ones_d = nc.declare_dram_parameter("ones", [P, H], F32, isOutput=False)
    out_d = nc.declare_dram_parameter("out", [S, D], F32, isOutput=True)

    with tile.TileContext(nc) as tc:
        with (
            tc.tile_pool(name="res", bufs=1) as res,          # resident tensors
            tc.tile_pool(name="wstream", bufs=4) as wstream,  # weight tiles
            tc.tile_pool(name="wstream2", bufs=1) as wstream2,
            tc.tile_pool(name="temps", bufs=4) as temps,
            tc.tile_pool(name="attn", bufs=3) as attnp,
            tc.tile_pool(name="quvp", bufs=3) as quvp,
            tc.tile_pool(name="khpp", bufs=3) as khpp,
            tc.tile_pool(name="outp", bufs=2) as outp,
            tc.tile_pool(name="small", bufs=2) as small,
            tc.tile_pool(name="bcp", bufs=2) as bcp,
            tc.tile_pool(name="pp", bufs=3, space="PSUM") as pp,
            tc.tile_pool(name="stp", bufs=2, space="PSUM") as stp,
            tc.tile_pool(name="avp", bufs=2, space="PSUM") as avp,
        ):
            # ---- resident loads ----
            xT = []
            for kc in range(KC):
                t = res.tile([P, S], F32R, tag=f"xT{kc}")
                nc.sync.dma_start(t[:], xT_d[kc * P:(kc + 1) * P, :].bitcast(F32R))
                xT.append(t)
            posT = []
            for kc in range(KC):
                t = res.tile([P, S], F32R, tag=f"posT{kc}")
                nc.sync.dma_start(t[:], posT_d[kc * P:(kc + 1) * P, :].bitcast(F32R))
                posT.append(t)
            ucol = res.tile([P, NMB], F32, tag="ucol")
            nc.sync.dma_start(ucol[:], ucol_d[:].rearrange("(mb p) -> p mb", p=P))
            vcol = res.tile([P, NMB], F32, tag="vcol")
            nc.sync.dma_start(vcol[:], vcol_d[:].rearrange("(mb p) -> p mb", p=P))
            mpats = []
            for i in range(n_pats):
                t = res.tile([P, P], F32, tag=f"mpat{i}")
                nc.sync.dma_start(t[:], mp_d[i, :, :])
                mpats.append(t)

            # ---- v-projection: v'[sb] = (s-part 128, [h, 65]) with ones col
            vprime = []
            for sb in range(NJT):
                t = res.tile([P, H, DH + 1], F32R, tag=f"vp{sb}")
                nc.sync.dma_start(
                    t[:, :, DH:DH + 1],
                    ones_d[:, :].bitcast(F32R).unsqueeze(2),
                )
                vprime.append(t)
            for nch in range(NSC):
                wv_t = wstream2.tile([P, KC, SC], F32R, tag="wvo")
                nc.sync.dma_start(
                    wv_t[:],
                    wv_d[:].bitcast(F32R)
                    .rearrange("(kc p) m -> p kc m", p=P)[:, :, nch * SC:(nch + 1) * SC],
                )
                for sb in range(NJT):
                    ps = pp.tile([P, SC], F32, tag="pp")
                    for kc in range(KC):
                        nc.tensor.matmul(
                            ps[:],
                            xT[kc][:, sb * P:(sb + 1) * P],
                            wv_t[:, kc, :],
                            start=(kc == 0),
                            stop=(kc == KC - 1),
                        )
                    # evac into interleaved v' layout (8 heads per n-chunk)
                    nc.scalar.copy(
                        vprime[sb][:, nch * 8:(nch + 1) * 8, 0:DH],
                        ps[:].rearrange("p (h d) -> p h d", d=DH),
                    )

            # ---- awvT tiles (filled per head-pair below)
            awvT = [
                res.tile([P, S], F32R, tag=f"awvT{mb}", name=f"awvT{mb}")
                for mb in range(NMB)
            ]

            # ---- head-pair loop: q/k/p projections then attention
            for mb in range(NMB):
                h_e, h_o = 2 * mb, 2 * mb + 1
                quv = {
                    h_e: quvp.tile([P, S], F32R, tag="quv", name=f"quv{h_e}"),
                    h_o: quvp.tile([P, S], F32R, tag="quv", name=f"quv{h_o}"),
                }
                khp = {
                    h_e: khpp.tile([P, S], F32R, tag="khp", name=f"khp{h_e}"),
                    h_o: khpp.tile([P, S], F32R, tag="khp", name=f"khp{h_o}"),
                }

                wq_t = wstream.tile([P, KC, P], F32R, tag="wqkp")
                nc.sync.dma_start(
                    wq_t[:],
                    wq_d[:].bitcast(F32R)
                    .rearrange("(kc p) m -> p kc m", p=P)[:, :, mb * P:(mb + 1) * P],
                )
                wk_t = wstream.tile([P, KC, P], F32R, tag="wqkp")
                nc.sync.dma_start(
                    wk_t[:],
                    wk_d[:].bitcast(F32R)
                    .rearrange("(kc p) m -> p kc m", p=P)[:, :, mb * P:(mb + 1) * P],
                )
                wp_t = wstream.tile([P, KC, P], F32R, tag="wqkp")
                nc.sync.dma_start(
                    wp_t[:],
                    wp_d[:].bitcast(F32R)
                    .rearrange("(kc p) m -> p kc m", p=P)[:, :, mb * P:(mb + 1) * P],
                )

                for sc in range(NSC):
                    ssl = slice(sc * SC, (sc + 1) * SC)
                    # q: psum rows 0:64 = head h_e, 64:128 = head h_o
                    ps = pp.tile([P, SC], F32, tag="pp")
                    for kc in range(KC):
                        nc.tensor.matmul(ps[:], wq_t[:, kc, :], xT[kc][:, ssl],
                                         start=(kc == 0), stop=(kc == KC - 1))
                    # aligned halves directly (+u / +v bias per partition)
                    nc.scalar.activation(
                        quv[h_e][0:DH, ssl], ps[0:DH, :],
                        mybir.ActivationFunctionType.Identity,
                        bias=ucol[0:DH, mb:mb + 1])
                    nc.scalar.activation(
                        quv[h_o][DH:P, ssl], ps[DH:P, :],
                        mybir.ActivationFunctionType.Identity,
                        bias=vcol[DH:P, mb:mb + 1])
                    # cross-partition halves: ACT to temp, then SBUF->SBUF DMA
                    qo_t = temps.tile([P, SC], F32R, tag="evac")  # rows 64:128 (+u)
                    nc.scalar.activation(
                        qo_t[DH:P, :], ps[DH:P, :],
                        mybir.ActivationFunctionType.Identity,
                        bias=ucol[DH:P, mb:mb + 1])
                    qe_t = temps.tile([P, SC], F32R, tag="evac")  # rows 0:64 (+v)
                    nc.scalar.activation(
                        qe_t[0:DH, :], ps[0:DH, :],
                        mybir.ActivationFunctionType.Identity,
                        bias=vcol[0:DH, mb:mb + 1])
                    nc.sync.dma_start(quv[h_o][0:DH, ssl], qo_t[DH:P, :])
                    nc.sync.dma_start(quv[h_e][DH:P, ssl], qe_t[0:DH, :])

                    # k: khp rows 0:64
                    ps = pp.tile([P, SC], F32, tag="pp")
                    for kc in range(KC):
                        nc.tensor.matmul(ps[:], wk_t[:, kc, :], xT[kc][:, ssl],
                                         start=(kc == 0), stop=(kc == KC - 1))
                    nc.scalar.copy(khp[h_e][0:DH, ssl], ps[0:DH, :])
                    ko_t = temps.tile([P, SC], F32R, tag="evac")
                    nc.scalar.copy(ko_t[DH:P, :], ps[DH:P, :])
                    nc.sync.dma_start(khp[h_o][0:DH, ssl], ko_t[DH:P, :])

                    # p: khp rows 64:128
                    ps = pp.tile([P, SC], F32, tag="pp")
                    for kc in range(KC):
                        nc.tensor.matmul(ps[:], wp_t[:, kc, :], posT[kc][:, ssl],
                                         start=(kc == 0), stop=(kc == KC - 1))
                    nc.scalar.copy(khp[h_o][DH:P, ssl], ps[DH:P, :])
                    pe_t = temps.tile([P, SC], F32R, tag="evac")
                    nc.scalar.copy(pe_t[0:DH, :], ps[0:DH, :])
                    nc.sync.dma_start(khp[h_e][DH:P, ssl], pe_t[0:DH, :])

                # ---- attention for heads h_e, h_o
                for h in (h_e, h_o):
                    for c in range(NSC):
                        isl = slice(c * SC, (c + 1) * SC)
                        jts = struct[c]
                        av = avp.tile([DH + 1, SC], F32, tag="av")
                        if not jts:
                            nc.vector.memset(av[:], 0.0)
                        for idx, (jt, ops) in enumerate(jts):
                            st = stp.tile([P, SC], F32, tag="st")
                            nc.tensor.matmul(
                                st[:], khp[h][:, jt * P:(jt + 1) * P],
                                quv[h][:, isl], start=True, stop=True)
                            for il, pi in ops:
                                nc.vector.tensor_add(
                                    st[:, il * P:(il + 1) * P],
                                    st[:, il * P:(il + 1) * P],
                                    mpats[pi][:])
                            at = attnp.tile([P, SC], F32R, tag="at")
                            nc.scalar.activation(
                                at[:], st[:],
                                mybir.ActivationFunctionType.Exp,
                                scale=SCALE)
                            nc.tensor.matmul(
                                av[:], vprime[jt][:, h, :], at[:],
                                start=(idx == 0), stop=(idx == len(jts) - 1))
                        # normalize rows 0:64 by row 64 (denominator)
                        rec = small.tile([1, SC], F32, tag="rec")
                        nc.vector.reciprocal(rec[:], av[DH:DH + 1, :])
                        bc = bcp.tile([DH, SC], F32, tag="bc")
                        nc.gpsimd.partition_broadcast(bc[:], rec[:])
                        nc.vector.tensor_mul(
                            awvT[mb][(h % 2) * DH:(h % 2 + 1) * DH, isl],
                            av[0:DH, :], bc[:])

            # ---- output projection: out = awvT.T @ W_out
            for nch in range(NSC):
                wo_t = wstream2.tile([P, KC, SC], F32R, tag="wvo")
                nc.sync.dma_start(
                    wo_t[:],
                    wo_d[:].bitcast(F32R)
                    .rearrange("(kc p) m -> p kc m", p=P)[:, :, nch * SC:(nch + 1) * SC],
                )
                for st_i in range(NJT):
                    ps = pp.tile([P, SC], F32, tag="pp")
                    for kc in range(KC):
                        nc.tensor.matmul(
                            ps[:], awvT[kc][:, st_i * P:(st_i + 1) * P],
                            wo_t[:, kc, :],
                            start=(kc == 0), stop=(kc == KC - 1))
                    ot = outp.tile([P, SC], F32, tag="ot")
                    nc.scalar.copy(ot[:], ps[:])
                    nc.sync.dma_start(
                        out_d[st_i * P:(st_i + 1) * P, nch * SC:(nch + 1) * SC],
                        ot[:])

    nc.compile()
    return nc


def _get_kernel(mask: np.ndarray):
    struct, patterns = _analyze_mask(mask)
    key = (struct, patterns.shape[0], patterns.tobytes())
    if key not in _BUILD_CACHE:
        _BUILD_CACHE[key] = (_build(struct, patterns.shape[0]), patterns)
    return _BUILD_CACHE[key]


def kernel(input_, pos_embs, u, v, W_kv, W_q, W_p, W_out, mask, _want_results=False):
    input_ = np.asarray(input_, dtype=np.float32)
    pos_embs = np.asarray(pos_embs, dtype=np.float32)
    u = np.asarray(u, dtype=np.float32)
    v = np.asarray(v, dtype=np.float32)
    W_kv = np.asarray(W_kv, dtype=np.float32)
    W_q = np.asarray(W_q, dtype=np.float32)
    W_p = np.asarray(W_p, dtype=np.float32)
    W_out = np.asarray(W_out, dtype=np.float32)

    nc, patterns = _get_kernel(np.asarray(mask))

    posT = np.ascontiguousarray(pos_embs[:, 0, :].T)
    Wk = np.ascontiguousarray(W_kv[:, : H * DH])
    Wv = np.ascontiguousarray(W_kv[:, H * DH:])
    ucol = np.ascontiguousarray(u.reshape(-1))
    vcol = np.ascontiguousarray(v.reshape(-1))

    in_maps = []
    for b in range(B):
        in_maps.append({
            "xT": np.ascontiguousarray(input_[:, b, :].T),
            "posT": posT,
            "Wq": W_q,
            "Wk": Wk,
            "Wv": Wv,
            "Wp": W_p,
            "Wout": W_out,
            "ucol": ucol,
            "vcol": vcol,
            "mpats": patterns,
            "ones": _ONES,
        })

    res = run_bass_kernel_spmd(nc, in_maps, list(range(B)))
    out = np.stack([res.results[b]["out"] for b in range(B)], axis=1)
    if _want_results:
        return out, res
    return out


# revision 13
# speedup vs baseline: 1.1241x; 1.1241x over previous
"""Trainium2 Bass kernel for MultiHeadAttentionXL (Transformer-XL attention).

Problem: input_ (S=1024, B=8, D=1024), 16 heads x 64 dim, causal mask.
Sharding: data-parallel over batch -- one batch element per NeuronCore (8 cores).

Per-core algorithm (all matmuls f32r: 1 cycle/row on PE, ~1e-4 accuracy):
  xT (D, S) resident in SBUF (host pre-transposed).
  v = x @ W_v             -> v' tiles (s-part, [16 heads x (64 d + ones col)])
  q = x @ W_q  (as qT)    -> quv_h tiles (128 = [q_h+u; q_h+v], S) via psum evac
  k = x @ W_k  (as kT)    -> khp_h tiles rows 0:64
  p = pos @ W_p (as pT)   -> khp_h tiles rows 64:128
  scores^T[j,i] = khp_h[:,j].T @ quv_h[:,i]   (one K=128 matmul per j-tile)
  attnT = exp(SCALE * scores^T + mask)        (ACT, additive mask on diag blocks)
  avT[dd,i]   = v'_h[j,dd].T @ attnT[j,i]     (accumulated over j-tiles; row 64
                                               of v' is ones -> denominator row)
  awvT = avT[0:64] * recip(avT[64])           (batched approx reciprocal)
  out = awvT.T @ W_out
Block skipping, per-j-tile valid-i ranges, and additive-mask patterns are all
derived from the actual mask tensor at compile time (optimal for causal,
correct for any mask).
"""

import sys

if "/opt/trn_rl_repo" not in sys.path:
    sys.path.insert(0, "/opt/trn_rl_repo")

import numpy as np

import concourse.bass as bass
import concourse.mybir as mybir
import concourse.tile as tile
from concourse import bacc
from concourse.bass_utils import run_bass_kernel_spmd

S = 1024
B = 8
D = 1024
H = 16
DH = 64
SCALE = 1.0 / (DH**0.5)
P = 128            # partitions / tile edge
KC = D // P        # 8 contraction chunks
NMB = H // 2       # 8 head-pairs (m-blocks of 128)
NSC = 2            # i-chunks of 512
SC = 512
NJT = S // P       # 8 j-tiles
NEG = -1e30

F32 = mybir.dt.float32
F32R = mybir.dt.float32r
Act = mybir.ActivationFunctionType

_BUILD_CACHE: dict = {}
_ONES = np.ones((128, 16), np.float32)


def _analyze_mask(mask: np.ndarray):
    """Derive block structure from the (S, S, 1) bool mask.

    Returns (struct, patterns): struct[c] is a tuple of
    (jt, off_cols, ((il, pat_idx), ...)) for each non-fully-masked j-tile of
    chunk c; off_cols is the width of the leading fully-masked block prefix
    (skipped in scores/exp/AV -- forced to 0 for the first j-tile so the
    PSUM accumulation covers the whole chunk). patterns is (NP, 128, 128)
    f32 of unique additive mask blocks in (j, i) orientation.
    """
    m = np.asarray(mask).reshape(S, S)
    pat_map: dict[bytes, int] = {}
    patterns: list[np.ndarray] = []

    def pat_idx(blk):
        pat = (NEG * blk.T).astype(np.float32)  # (j, i) orientation
        key = pat.tobytes()
        if key not in pat_map:
            pat_map[key] = len(patterns)
            patterns.append(pat)
        return pat_map[key]

    struct = []
    for c in range(NSC):
        jts = []
        for jt in range(NJT):
            blks = [
                m[(c * 4 + il) * P:(c * 4 + il + 1) * P, jt * P:(jt + 1) * P]
                for il in range(4)
            ]
            if all(b.all() for b in blks):
                continue  # whole j-tile masked for this chunk: skip
            first = len(jts) == 0
            off = 0
            if not first:
                while off < 3 and blks[off].all():
                    off += 1
            ops = tuple(
                (il, pat_idx(blks[il]))
                for il in range(off, 4)
                if blks[il].any()
            )
            jts.append((jt, off * P, ops))
        struct.append(tuple(jts))
    if not patterns:
        patterns.append(np.zeros((P, P), np.float32))
    return tuple(struct), np.stack(patterns)


def _build(struct, n_pats: int):
    nc = bacc.Bacc("TRN2", target_bir_lowering=False, debug=False, num_devices=B,
                   dynamic_dma_scratch_size=2048)

    xT_d = nc.declare_dram_parameter("xT", [D, S], F32, isOutput=False)
    posT_d = nc.declare_dram_parameter("posT", [D, S], F32, isOutput=False)
    wq_d = nc.declare_dram_parameter("Wq", [D, H * DH], F32, isOutput=False)
    wk_d = nc.declare_dram_parameter("Wk", [D, H * DH], F32, isOutput=False)
    wv_d = nc.declare_dram_parameter("Wv", [D, H * DH], F32, isOutput=False)
    wp_d = nc.declare_dram_parameter("Wp", [D, H * DH], F32, isOutput=False)
    wo_d = nc.declare_dram_parameter("Wout", [H * DH, D], F32, isOutput=False)
    ucol_d = nc.declare_dram_parameter("ucol", [H * DH], F32, isOutput=False)
    vcol_d = nc.declare_dram_parameter("vcol", [H * DH], F32, isOutput=False)
    mp_d = nc.declare_dram_parameter("mpats", [n_pats, P, P], F32, isOutput=False)
    ones_d = nc.declare_dram_parameter("ones", [P, H], F32, isOutput=False)
    out_d = nc.declare_dram_parameter("out", [S, D], F32, isOutput=True)

    def rearr(w):
        return w[:].bitcast(F32R).rearrange("(kc p) m -> p kc m", p=P)

    with tile.TileContext(nc) as tc:
        with (
            tc.tile_pool(name="res", bufs=1) as res,          # resident tensors
            tc.tile_pool(name="wstream", bufs=4) as wstream,  # weight tiles
            tc.tile_pool(name="wstream2", bufs=1) as wstream2,
            tc.tile_pool(name="temps", bufs=6) as temps,
            tc.tile_pool(name="attn", bufs=3) as attnp,
            tc.tile_pool(name="quvp", bufs=3) as quvp,
            tc.tile_pool(name="khpp", bufs=3) as khpp,
            tc.tile_pool(name="outp", bufs=2) as outp,
            tc.tile_pool(name="bcp", bufs=2) as bcp,
            tc.tile_pool(name="pp", bufs=3, space="PSUM") as pp,
            tc.tile_pool(name="stp", bufs=2, space="PSUM") as stp,
            tc.tile_pool(name="avp", bufs=2, space="PSUM") as avp,
        ):
            # ---- resident loads ----
            xT = []
            for kc in range(KC):
                t = res.tile([P, S], F32R, tag=f"xT{kc}", name=f"xT{kc}")
                nc.sync.dma_start(t[:], xT_d[kc * P:(kc + 1) * P, :].bitcast(F32R))
                xT.append(t)
            posT = []
            for kc in range(KC):
                t = res.tile([P, S], F32R, tag=f"posT{kc}", name=f"posT{kc}")
                nc.sync.dma_start(t[:], posT_d[kc * P:(kc + 1) * P, :].bitcast(F32R))
                posT.append(t)
            ucol = res.tile([P, NMB], F32, tag="ucol")
            nc.sync.dma_start(ucol[:], ucol_d[:].rearrange("(mb p) -> p mb", p=P))
            vcol = res.tile([P, NMB], F32, tag="vcol")
            nc.sync.dma_start(vcol[:], vcol_d[:].rearrange("(mb p) -> p mb", p=P))
            mpats = []
            for i in range(n_pats):
                t = res.tile([P, P], F32, tag=f"mpat{i}", name=f"mpat{i}")
                nc.sync.dma_start(t[:], mp_d[i, :, :])
                mpats.append(t)

            # ---- v-projection: v'[sb] = (s-part 128, [h, 65]) with ones col
            vprime = []
            for sb in range(NJT):
                t = res.tile([P, H, DH + 1], F32R, tag=f"vp{sb}", name=f"vp{sb}")
                nc.sync.dma_start(
                    t[:, :, DH:DH + 1],
                    ones_d[:, :].bitcast(F32R).unsqueeze(2),
                )
                vprime.append(t)
            for nch in range(NSC):
                wv_t = wstream2.tile([P, KC, SC], F32R, tag="wvo", name="wv_t")
                nc.sync.dma_start(
                    wv_t[:], rearr(wv_d)[:, :, nch * SC:(nch + 1) * SC])
                for sb in range(NJT):
                    ps = pp.tile([P, SC], F32, tag="pp")
                    for kc in range(KC):
                        nc.tensor.matmul(
                            ps[:],
                            xT[kc][:, sb * P:(sb + 1) * P],
                            wv_t[:, kc, :],
                            start=(kc == 0),
                            stop=(kc == KC - 1),
                        )
                    # evac into interleaved v' layout (8 heads per n-chunk)
                    nc.scalar.copy(
                        vprime[sb][:, nch * 8:(nch + 1) * 8, 0:DH],
                        ps[:].rearrange("p (h d) -> p h d", d=DH),
                    )

            # ---- awvT tiles (filled per head-pair below)
            awvT = [
                res.tile([P, S], F32R, tag=f"awvT{mb}", name=f"awvT{mb}")
                for mb in range(NMB)
            ]

            # ---- head-pair loop: q/k/p projections then attention
            for mb in range(NMB):
                h_e, h_o = 2 * mb, 2 * mb + 1
                quv = {
                    h_e: quvp.tile([P, S], F32R, tag="quv", name=f"quv{h_e}"),
                    h_o: quvp.tile([P, S], F32R, tag="quv", name=f"quv{h_o}"),
                }
                khp = {
                    h_e: khpp.tile([P, S], F32R, tag="khp", name=f"khp{h_e}"),
                    h_o: khpp.tile([P, S], F32R, tag="khp", name=f"khp{h_o}"),
                }

                wq_t = wstream.tile([P, KC, P], F32R, tag="wqkp", name=f"wq{mb}")
                nc.sync.dma_start(wq_t[:], rearr(wq_d)[:, :, mb * P:(mb + 1) * P])
                wk_t = wstream.tile([P, KC, P], F32R, tag="wqkp", name=f"wk{mb}")
                nc.sync.dma_start(wk_t[:], rearr(wk_d)[:, :, mb * P:(mb + 1) * P])
                wp_t = wstream.tile([P, KC, P], F32R, tag="wqkp", name=f"wp{mb}")
                nc.sync.dma_start(wp_t[:], rearr(wp_d)[:, :, mb * P:(mb + 1) * P])

                for sc in range(NSC):
                    ssl = slice(sc * SC, (sc + 1) * SC)
                    # q: psum rows 0:64 = head h_e, 64:128 = head h_o
                    ps = pp.tile([P, SC], F32, tag="pp")
                    for kc in range(KC):
                        nc.tensor.matmul(ps[:], wq_t[:, kc, :], xT[kc][:, ssl],
                                         start=(kc == 0), stop=(kc == KC - 1))
                    # full-partition evacs (+u / +v bias), scatter via DMA
                    qu_t = temps.tile([P, SC], F32R, tag="evac", name="qu_t")
                    nc.scalar.activation(qu_t[:], ps[:], Act.Identity,
                                         bias=ucol[:, mb:mb + 1])
                    qv_t = temps.tile([P, SC], F32R, tag="evac", name="qv_t")
                    nc.scalar.activation(qv_t[:], ps[:], Act.Identity,
                                         bias=vcol[:, mb:mb + 1])
                    nc.sync.dma_start(quv[h_e][0:DH, ssl], qu_t[0:DH, :])
                    nc.sync.dma_start(quv[h_o][0:DH, ssl], qu_t[DH:P, :])
                    nc.sync.dma_start(quv[h_e][DH:P, ssl], qv_t[0:DH, :])
                    nc.sync.dma_start(quv[h_o][DH:P, ssl], qv_t[DH:P, :])

                    # k: khp rows 0:64
                    ps = pp.tile([P, SC], F32, tag="pp")
                    for kc in range(KC):
                        nc.tensor.matmul(ps[:], wk_t[:, kc, :], xT[kc][:, ssl],
                                         start=(kc == 0), stop=(kc == KC - 1))
                    kt_t = temps.tile([P, SC], F32R, tag="evac", name="kt_t")
                    nc.scalar.copy(kt_t[:], ps[:])
                    nc.sync.dma_start(khp[h_e][0:DH, ssl], kt_t[0:DH, :])
                    nc.sync.dma_start(khp[h_o][0:DH, ssl], kt_t[DH:P, :])

                    # p: khp rows 64:128
                    ps = pp.tile([P, SC], F32, tag="pp")
                    for kc in range(KC):
                        nc.tensor.matmul(ps[:], wp_t[:, kc, :], posT[kc][:, ssl],
                                         start=(kc == 0), stop=(kc == KC - 1))
                    pt_t = temps.tile([P, SC], F32R, tag="evac", name="pt_t")
                    nc.scalar.copy(pt_t[:], ps[:])
                    nc.sync.dma_start(khp[h_e][DH:P, ssl], pt_t[0:DH, :])
                    nc.sync.dma_start(khp[h_o][DH:P, ssl], pt_t[DH:P, :])

                # ---- attention for heads h_e, h_o
                for h in (h_e, h_o):
                    for c in range(NSC):
                        jts = struct[c]
                        av = avp.tile([DH + 1, SC], F32, tag="av")
                        if not jts:
                            nc.vector.memset(av[:], 0.0)
                        for idx, (jt, off, ops) in enumerate(jts):
                            st = stp.tile([P, SC], F32, tag="st")
                            nc.tensor.matmul(
                                st[:, off:], khp[h][:, jt * P:(jt + 1) * P],
                                quv[h][:, c * SC + off:(c + 1) * SC],
                                start=True, stop=True)
                            for il, pi in ops:
                                nc.vector.tensor_add(
                                    st[:, il * P:(il + 1) * P],
                                    st[:, il * P:(il + 1) * P],
                                    mpats[pi][:])
                            at = attnp.tile([P, SC], F32R, tag="at")
                            nc.scalar.activation(at[:, off:], st[:, off:],
                                                 Act.Exp, scale=SCALE)
                            nc.tensor.matmul(
                                av[:, off:], vprime[jt][:, h, :], at[:, off:],
                                start=(idx == 0), stop=(idx == len(jts) - 1))
                        # denominator: psum row -> partition 0 -> recip ->
                        # broadcast; then normalize during psum evac
                        dtmp = temps.tile([P, SC], F32, tag="evac", name="dtmp")
                        nc.scalar.copy(dtmp[DH:DH + 1, :], av[DH:DH + 1, :])
                        dn = bcp.tile([1, SC], F32, tag="dn", name="dn")
                        nc.sync.dma_start(dn[0:1, :], dtmp[DH:DH + 1, :])
                        rn = bcp.tile([1, SC], F32, tag="rn", name="rn")
                        nc.vector.reciprocal_approx_fast(rn[0:1, :], dn[0:1, :])
                        bc = bcp.tile([DH, SC], F32, tag="bc", name="bc")
                        nc.gpsimd.partition_broadcast(bc[:], rn[0:1, :])
                        nc.vector.tensor_mul(
                            awvT[mb][(h % 2) * DH:(h % 2 + 1) * DH,
                                     c * SC:(c + 1) * SC],
                            av[0:DH, :], bc[:])

            # ---- output projection: out = awvT.T @ W_out
            for nch in range(NSC):
                wo_t = wstream2.tile([P, KC, SC], F32R, tag="wvo", name="wo_t")
                nc.sync.dma_start(
                    wo_t[:], wo_d[:].bitcast(F32R)
                    .rearrange("(kc p) m -> p kc m", p=P)[:, :, nch * SC:(nch + 1) * SC])
                for st_i in range(NJT):
                    ps = pp.tile([P, SC], F32, tag="pp")
                    for kc in range(KC):
                        nc.tensor.matmul(
                            ps[:], awvT[kc][:, st_i * P:(st_i + 1) * P],
                            wo_t[:, kc, :],
                            start=(kc == 0), stop=(kc == KC - 1))
                    ot = outp.tile([P, SC], F32, tag="ot")
                    nc.scalar.copy(ot[:], ps[:])
                    nc.sync.dma_start(
                        out_d[st_i * P:(st_i + 1) * P, nch * SC:(nch + 1) * SC],
                        ot[:])

    nc.compile()
    return nc


def _get_kernel(mask: np.ndarray):
    struct, patterns = _analyze_mask(mask)
    key = (struct, patterns.shape[0], patterns.tobytes())
    if key not in _BUILD_CACHE:
        _BUILD_CACHE[key] = (_build(struct, patterns.shape[0]), patterns)
    return _BUILD_CACHE[key]


def kernel(input_, pos_embs, u, v, W_kv, W_q, W_p, W_out, mask, _want_results=False):
    input_ = np.asarray(input_, dtype=np.float32)
    pos_embs = np.asarray(pos_embs, dtype=np.float32)
    u = np.asarray(u, dtype=np.float32)
    v = np.asarray(v, dtype=np.float32)
    W_kv = np.asarray(W_kv, dtype=np.float32)
    W_q = np.asarray(W_q, dtype=np.float32)
    W_p = np.asarray(W_p, dtype=np.float32)
    W_out = np.asarray(W_out, dtype=np.float32)

    nc, patterns = _get_kernel(np.asarray(mask))

    posT = np.ascontiguousarray(pos_embs[:, 0, :].T)
    Wk = np.ascontiguousarray(W_kv[:, : H * DH])
    Wv = np.ascontiguousarray(W_kv[:, H * DH:])
    ucol = np.ascontiguousarray(u.reshape(-1))
    vcol = np.ascontiguousarray(v.reshape(-1))

    in_maps = []
    for b in range(B):
        in_maps.append({
            "xT": np.ascontiguousarray(input_[:, b, :].T),
            "posT": posT,
            "Wq": W_q,
            "Wk": Wk,
            "Wv": Wv,
            "Wp": W_p,
            "Wout": W_out,
            "ucol": ucol,
            "vcol": vcol,
            "mpats": patterns,
            "ones": _ONES,
        })

    res = run_bass_kernel_spmd(nc, in_maps, list(range(B)))
    out = np.stack([res.results[b]["out"] for b in range(B)], axis=1)
    if _want_results:
        return out, res
    return out
